# revision 30
# baseline (speedup 1.0000x reference)
"""DeltaImportance kernel for Trainium2 (8 NeuronCores, data-parallel over batch).

out[b,i,:] = |delta_mag[b,i] * (decay @ h[b])[i,:]|
  delta     = cumsum(h, T) - shift1(h)
  delta_mag = ||delta||_2 over D
  decay[i,j] = exp(-softplus(k)*(i-j)) * (j<=i)

Structure per core (one batch, [T=2048, D=512]):
  T is split into 16 chunks of C=128 rows (chunk rows on SBUF partitions).
  All HBM I/O is bf16 (rel-err budget 2e-2; bf16 costs ~0.5%): h is
  uploaded pre-transposed as [128, 16*512] so loads are few large DMAs,
  and out is stored bf16 in the same partition-major layout (host
  un-transposes + casts). Boundary rows h[n*C-1] are host-gathered and
  uploaded, removing the strided gather DMA.
  - context chunk n = sum_l DlT.T @ h_{n-l}  (banded: exp underflows
    beyond distance ~104/s, so only n_prev+1 lag tiles are nonzero)
  - delta chunk n = M0T.T @ h_n + K2T.T @ [R[n]; h[n*C-1]]
    where M0 = lower-tri ones with zero subdiagonal, R[n] = sum of all rows
    of chunks < n (masked-accumulation matmuls in PSUM), K2 = [ones; -e0].
  - delta_mag^2 = D*(mean^2+var) via DVE bn_stats (single PSUM read).
  - out chunk = |ctx|*mag fused into the PSUM evacuation (ACT Abs with
    per-partition scale; DVE abs_max/mult for a few chunks to balance),
    into a persistent SBUF tile; stores are batched 4 chunks per DMA
    (HWDGE is a shared serial unit at ~625ns per dma_start).
  Matmuls run in bfloat16 (1 cycle/row on PE, same as fp32r at N=512).
  Carries R[n] come from three masked-accumulation prefix groups
  (R[1..6], R[7..8], R[9..15]) chained by K=1 seed matmuls reading the
  SBUF copies directly; rows are flattened onto one partition by small
  SBUF->SBUF DMAs on the scalar-engine queue so the bulk load/store
  queue (sync engine) is never blocked.
"""

import sys

if "/opt/trn_rl_repo" not in sys.path:
    sys.path.insert(0, "/opt/trn_rl_repo")

import numpy as np
import ml_dtypes

import concourse.bass as bass
import concourse.bacc as bacc
import concourse.tile as tile
from bass_rust import add_dep_helper
from concourse import mybir
from concourse.bass_utils import run_bass_kernel_spmd

B, T, D = 8, 2048, 512
C = 128
NCH = T // C  # 16
G = 2         # chunks per DMA transfer (in)
NG = NCH // G
SG = 4        # chunks per DMA transfer (out)
FP32 = mybir.dt.float32
BF16 = mybir.dt.bfloat16
BF = ml_dtypes.bfloat16

_prog_cache = {}


def _build_program(n_prev):
    nc = bacc.Bacc(
        "TRN2", target_bir_lowering=False, debug=False, num_devices=B
    )
    LW = (n_prev + 1) * C  # decay lhsT width
    CW = LW + C + 36 + 4 + 49  # + m0 + uta + utc + utb
    h_d = nc.dram_tensor("h", [C, NCH * D], BF16, kind="ExternalInput").ap()
    cst_d = nc.dram_tensor("cst", [C, CW], BF16, kind="ExternalInput").ap()
    k2_d = nc.dram_tensor("k2", [33, C], BF16, kind="ExternalInput").ap()
    bnd_d = nc.dram_tensor(
        "bnd", [1, (NCH - 1) * D], BF16, kind="ExternalInput"
    ).ap()
    out_d = nc.dram_tensor(
        "out", [C, NCH * D], BF16, kind="ExternalOutput"
    ).ap()

    Sqrt = mybir.ActivationFunctionType.Sqrt
    Abs = mybir.ActivationFunctionType.Abs

    with tile.TileContext(nc) as tc:
        with (
            tc.tile_pool(name="const", bufs=1) as const,
            tc.tile_pool(name="hbuf", bufs=1) as hbuf,
            tc.tile_pool(name="rsb", bufs=1) as rsb,
            tc.tile_pool(name="small", bufs=12) as small,
            tc.tile_pool(name="outp", bufs=1) as outp,
            tc.tile_pool(name="psr", bufs=1, space="PSUM") as psr,
            tc.tile_pool(name="psc", bufs=3, space="PSUM") as psc,
            tc.tile_pool(name="psd", bufs=3, space="PSUM") as psd,
        ):
            cst_sb = const.tile([C, CW], BF16, tag="cst")
            # scalar queue: overlaps seq/bus with the first h load
            nc.scalar.dma_start(out=cst_sb[:], in_=cst_d)

            # trigger both ACT table-set loads while the engines are idle
            tiny = const.tile([1, 1], FP32, tag="tiny")
            nc.vector.memset(tiny[:], 1.0)
            nc.scalar.activation(out=tiny[:], in_=tiny[:], func=Abs)
            nc.scalar.activation(out=tiny[:], in_=tiny[:], func=Sqrt)

            dec_sb = cst_sb[:, 0:LW]
            m0_sb = cst_sb[:, LW:LW + C]
            uta_sb = cst_sb[:, LW + C:LW + C + 36]
            utc_sb = cst_sb[:, LW + C + 36:LW + C + 40]
            utb_sb = cst_sb[:, LW + C + 40:LW + C + 89]

            # rc_all holds per-chunk K=2 rhs: partition 0 = R[n] (carry),
            # partition 1 = h[n*C-1] (boundary row), free slot n.
            rc_all = rsb.tile([2, NCH, D], BF16, tag="rc")

            # prefix groups: A1 -> R[1..6], A2 -> R[7..8], B -> R[9..15]
            ra1_ps = psr.tile([6, D], FP32, tag="ra1")
            ra2_ps = psr.tile([2, D], FP32, tag="ra2")
            # B reuses A1's bank (A1 is released by its copy before the
            # first B mask runs)
            rb_ps = psr.tile([7, D], FP32, tag="ra1", name="rb_ps")

            ht = hbuf.tile([C, NCH, D], BF16, tag="h")
            obuf = outp.tile([C, NCH, D], BF16, tag="o")

            def h_sb(n):
                return ht[:, n, :]

            k2_sb = None
            del_tiles = {}
            rs = rsb.tile([71, D], BF16, tag="rs")

            for g in range(NG):
                src = h_d[:, g * G * D:(g + 1) * G * D]
                nc.sync.dma_start(
                    out=ht[:, g * G:(g + 1) * G, :],
                    in_=src.rearrange("p (c d) -> p c d", c=G),
                )
                if g == 0:
                    # small late-use transfers on the Pool/SWDGE queue:
                    # they bypass the shared HWDGE unit so the bulk h
                    # loads stream back-to-back
                    k2_sb = const.tile([33, C], BF16, tag="k2")
                    nc.gpsimd.dma_start(out=k2_sb[:], in_=k2_d)
                    nc.gpsimd.dma_start(
                        out=rc_all[1:2, 1:NCH, :],
                        in_=bnd_d.rearrange("p (c d) -> p c d", c=NCH - 1),
                    )
                if g == 3:
                    # A1 -> SBUF: rs[0]=R[6], rs[1:6]=R[1..5]; strip to rc
                    nc.vector.tensor_copy(rs[0:6, :], ra1_ps[:])
                    nc.gpsimd.dma_start(
                        out=rc_all[0:1, 1:6, :], in_=rs[1:6, :]
                    )
                    nc.gpsimd.dma_start(
                        out=rc_all[0:1, 6:7, :], in_=rs[0:1, :]
                    )
                if g == 4:
                    # seed A2 with R[6] read directly from rs[0] (base 0)
                    nc.tensor.matmul(
                        ra2_ps[:], k2_sb[0:1, 0:2], rs[0:1, :],
                        start=False, stop=True,
                    )
                    # A2 -> SBUF: rs[32]=R[8], rs[33]=R[7]; strip to rc
                    nc.vector.tensor_copy(rs[32:34, :], ra2_ps[:])
                    nc.gpsimd.dma_start(
                        out=rc_all[0:1, 7:8, :], in_=rs[33:34, :]
                    )
                    nc.gpsimd.dma_start(
                        out=rc_all[0:1, 8:9, :], in_=rs[32:33, :]
                    )
                for c in range(g * G, (g + 1) * G):
                    rhs = h_sb(c)
                    if c <= 5:
                        nc.tensor.matmul(
                            ra1_ps[:], uta_sb[:, c * 6:(c + 1) * 6], rhs,
                            start=(c == 0), stop=(c == 5),
                        )
                    if 6 <= c <= 7:
                        mm = nc.tensor.matmul(
                            ra2_ps[:], utc_sb[:, (c - 6) * 2:(c - 5) * 2],
                            rhs, start=(c == 6), stop=False,
                        )
                        last_a_mm = mm
                    if 8 <= c <= 14:
                        nc.tensor.matmul(
                            rb_ps[:], utb_sb[:, (c - 8) * 7:(c - 7) * 7],
                            rhs, start=(c == 8), stop=False,
                        )
            # M0 matmuls: emitted after the pair loop so the prefix
            # matmuls (which gate the carry chain) always win PE priority;
            # the scheduler backfills these into PE idle slots
            mag0 = None
            # emitted (= PSUM buffer rotation) order must match the
            # epilogue consumption order below
            for c in [0] + list(range(1, 9)) + list(range(NCH - 1, 8, -1)):
                del_ps = psd.tile([C, D], FP32, tag="del_ps", name=f"del{c}")
                mm = nc.tensor.matmul(
                    del_ps[:], m0_sb, h_sb(c), start=True, stop=(c == 0)
                )
                # scheduling-only edge: keep the PE clear for the prefix
                # matmuls (they gate the carry chain -> all delta epilogues)
                add_dep_helper(mm.ins, last_a_mm.ins,
                               sync=False, reason="prefix-A first")
                del_tiles[c] = del_ps
                if c == 0:
                    # chunk 0 has no carry: take its delta_mag right away
                    # so its del PSUM buffer frees early; its ctx/evac/
                    # store run LAST (shallow tail: no K2->ttr chain there)
                    stats = small.tile([C, 6], FP32, tag="stats")
                    nc.vector.bn_stats(stats[:], del_ps[:])
                    mv = small.tile([C, 2], FP32, tag="mv")
                    nc.vector.bn_aggr(mv[:], stats[:])
                    ex2 = small.tile([C, 1], FP32, tag="ex2")
                    nc.vector.tensor_mul(ex2[:], mv[:, 0:1], mv[:, 0:1])
                    nc.vector.tensor_add(ex2[:], ex2[:], mv[:, 1:2])
                    mag0 = small.tile([C, 1], FP32, tag="mag0")
                    nc.scalar.activation(
                        out=mag0[:], in_=ex2[:], func=Sqrt, scale=float(D)
                    )

            # seed B with R[8] read directly from rs[32] (base 32)
            nc.tensor.matmul(
                rb_ps[:], k2_sb[32:33, 0:7], rs[32:33, :],
                start=False, stop=True,
            )
            nc.vector.tensor_copy(rs[64:71, :], rb_ps[:])
            nc.gpsimd.dma_start(out=rc_all[0:1, 9:16, :], in_=rs[64:71, :])

            # single interleaved loop: the carry-gated delta chain (K2 ->
            # ttr -> sqrt) and the carry-independent context chain
            # (ctx matmuls -> fused evac) are emitted chunk-interleaved so
            # each engine always has lower-priority ready work while the
            # carry is in flight
            # processing order: A1/A2 chunks 1-8 in order, then the B
            # group REVERSED (15 first) so chunk 15's K2->ttr->evac chain
            # clears early; chunk 0 last (its mag is ready from the start,
            # so the final tail is just ctx_0 -> evac -> 1-chunk store).
            # The last-processed quadruple (8-11) is stored as 2+2 so the
            # final bus transfer is small.
            STORES = {3: (1, 3), 7: (4, 4), 12: (12, 4), 10: (10, 2),
                      9: (8, 2), 0: (0, 1)}
            for n in list(range(1, 9)) + list(range(NCH - 1, 8, -1)) + [0]:
                del_ps = del_tiles[n]
                if n > 0:
                    nc.tensor.matmul(
                        del_ps[:], k2_sb[0:2, :], rc_all[:, n, :],
                        start=False, stop=True,
                    )
                    # delta_mag^2 = sum delta^2: square in place (delta
                    # is dead after), accumulate along free into sumsq
                    stats = small.tile([C, 6], FP32, tag="stats")
                    nc.vector.bn_stats(stats[:], del_ps[:])
                    mv = small.tile([C, 2], FP32, tag="mv")
                    nc.vector.bn_aggr(mv[:], stats[:])
                    ex2 = small.tile([C, 1], FP32, tag="ex2")
                    nc.vector.tensor_mul(ex2[:], mv[:, 0:1], mv[:, 0:1])
                    nc.vector.tensor_add(ex2[:], ex2[:], mv[:, 1:2])
                    mag = small.tile([C, 1], FP32, tag="mag")
                    nc.scalar.activation(
                        out=mag[:], in_=ex2[:], func=Sqrt, scale=float(D)
                    )
                else:
                    mag = mag0
                # context: banded decay matmuls + fused |ctx|*mag evac
                ctx_ps = psc.tile([C, D], FP32, tag="ctx_ps")
                lags = list(range(0, min(n, n_prev) + 1))
                for l in lags:
                    mm = nc.tensor.matmul(
                        ctx_ps[:], dec_sb[:, l * C:(l + 1) * C], h_sb(n - l),
                        start=(l == lags[0]), stop=(l == lags[-1]),
                    )
                    add_dep_helper(mm.ins, last_a_mm.ins,
                                   sync=False, reason="prefix first")
                o4 = obuf[:, n, :]
                nc.scalar.activation(
                    out=o4, in_=ctx_ps[:], func=Abs, scale=mag[:, 0:1]
                )
                if n in STORES:
                    s0, sn = STORES[n]
                    dst = out_d[:, s0 * D:(s0 + sn) * D]
                    nc.sync.dma_start(
                        out=dst.rearrange("p (c d) -> p c d", c=sn),
                        in_=obuf[:, s0:s0 + sn, :],
                    )

    nc.compile()
    return nc


def _host_consts(kval):
    # softplus in fp32, matching jax.nn.softplus numerics closely
    kf = np.float32(np.asarray(kval))
    s = np.float32(np.logaddexp(np.float32(0.0), kf))
    # distance beyond which exp(-s*d) is exactly 0 in fp32 (incl. subnormals)
    dmax = 103.98 / float(s)
    n_prev = int(min(NCH - 1, (dmax + C - 1) // C))

    i = np.arange(C)
    j = np.arange(C)
    dec = np.zeros((C, (n_prev + 1) * C), dtype=np.float32)
    for l in range(n_prev + 1):
        dist = (l * C + i[None, :] - j[:, None]).astype(np.float32)
        mask = dist >= 0
        tilew = np.exp((-s) * np.maximum(dist, 0)).astype(np.float32)
        tilew = np.where(mask, tilew, np.float32(0))  # [j, i]
        dec[:, l * C:(l + 1) * C] = tilew

    m0 = ((j[:, None] <= i[None, :]) & (j[:, None] != i[None, :] - 1)).astype(
        np.float32
    )  # [j, i] lhsT layout

    k2 = np.zeros((33, C), dtype=np.float32)
    k2[0, :] = 1.0
    k2[1, 0] = -1.0
    k2[32, :] = 1.0  # ones lhsT at base partition 32 (B seed)

    # prefix groups chained by K=1 seeds:
    # A1 rows R[1..6]; A2 rows R[7..8] = R[6] + partial; B rows R[9..15]
    # = R[8] + partial
    # A1 rows: m=0 -> R[6], m=1..5 -> R[1..5] (R[6] at partition 0 so the
    # A2 seed can read it straight from the SBUF copy)
    uta = np.zeros((C, 6 * 6), dtype=np.float32)
    for c in range(6):
        for m in range(6):
            r = 6 if m == 0 else m  # row m holds R[r] = sum_{c<r} S[c]
            if c < r:
                uta[:, c * 6 + m] = 1.0
    # A2 rows: m=0 -> R[8], m=1 -> R[7] (R[8] at partition 32 of the copy)
    utc = np.zeros((C, 2 * 2), dtype=np.float32)
    for ci, c in enumerate((6, 7)):
        for m in range(2):
            r = 8 - m
            if c < r:  # plus seed R[6]
                utc[:, ci * 2 + m] = 1.0
    utb = np.zeros((C, 7 * 7), dtype=np.float32)
    for ci, c in enumerate(range(8, 15)):
        for m in range(7):
            if c <= 8 + m:  # R[9+m] = R[8] + sum_{8<=c<=8+m} S[c]
                utb[:, ci * 7 + m] = 1.0

    cst = np.concatenate([dec, m0, uta, utc, utb], axis=1)
    return n_prev, {"cst": cst.astype(BF), "k2": k2.astype(BF)}


def _run(h, k, trace=False):
    n_prev, consts = _host_consts(k)
    if n_prev not in _prog_cache:
        _prog_cache[n_prev] = _build_program(n_prev)
    nc = _prog_cache[n_prev]
    h = np.ascontiguousarray(np.asarray(h, dtype=np.float32))
    hb = h.astype(BF)  # [B, T, D] bf16
    # pre-transpose to partition-major [C, NCH*D] per batch
    ht = np.ascontiguousarray(
        hb.reshape(B, NCH, C, D).transpose(0, 2, 1, 3).reshape(B, C, NCH * D)
    )
    # boundary rows h[n*C-1] for n=1..NCH-1, flattened on one partition
    bnd = np.ascontiguousarray(
        hb[:, C - 1:T - 1:C, :].reshape(B, 1, (NCH - 1) * D)
    )
    in_maps = [
        {"h": ht[b], "bnd": bnd[b], **consts} for b in range(B)
    ]
    res = run_bass_kernel_spmd(nc, in_maps, list(range(B)), trace=trace)
    out = np.stack(
        [
            res.results[b]["out"]
            .reshape(C, NCH, D)
            .transpose(1, 0, 2)
            .reshape(T, D)
            .astype(np.float32)
            for b in range(B)
        ],
        axis=0,
    )
    return out, res


def kernel(h, k):
    out, _ = _run(h, k, trace=False)
    return out


# revision 47
# speedup vs baseline: 1.0850x; 1.0850x over previous
"""DeltaImportance kernel for Trainium2 (8 NeuronCores, data-parallel over batch).

out[b,i,:] = |delta_mag[b,i] * (decay @ h[b])[i,:]|
  delta     = cumsum(h, T) - shift1(h)
  delta_mag = ||delta||_2 over D
  decay[i,j] = exp(-softplus(k)*(i-j)) * (j<=i)

Structure per core (one batch, [T=2048, D=512]):
  T is split into 16 chunks of C=128 rows (chunk rows on SBUF partitions).
  All HBM I/O is bf16 (rel-err budget 2e-2; bf16 costs ~0.5%): h is
  uploaded pre-transposed as [128, 16*512] so loads are few large DMAs,
  and out is stored bf16 in the same partition-major layout (host
  un-transposes + casts). Boundary rows h[n*C-1] are host-gathered and
  uploaded, removing the strided gather DMA.
  - context chunk n = sum_l DlT.T @ h_{n-l}  (banded: exp underflows
    beyond distance ~104/s, so only n_prev+1 lag tiles are nonzero)
  - delta chunk n = M0T.T @ h_n + K2T.T @ [R[n]; h[n*C-1]]
    where M0 = lower-tri ones with zero subdiagonal, R[n] = sum of all rows
    of chunks < n (masked-accumulation matmuls in PSUM), K2 = [ones; -e0].
  - delta_mag^2 = D*(mean^2+var) via DVE bn_stats (single PSUM read).
  - out chunk = |ctx|*mag fused into the PSUM evacuation (ACT Abs with
    per-partition scale; DVE abs_max/mult for a few chunks to balance),
    into a persistent SBUF tile; stores are batched 4 chunks per DMA
    (HWDGE is a shared serial unit at ~625ns per dma_start).
  Matmuls run in bfloat16 (1 cycle/row on PE, same as fp32r at N=512).
  Carries R[n] come from three masked-accumulation prefix groups
  (R[1..6], R[7..8], R[9..15]) chained by K=1 seed matmuls reading the
  SBUF copies directly; rows are flattened onto one partition by small
  SBUF->SBUF DMAs on the scalar-engine queue so the bulk load/store
  queue (sync engine) is never blocked.
"""

import sys

if "/opt/trn_rl_repo" not in sys.path:
    sys.path.insert(0, "/opt/trn_rl_repo")

import numpy as np
import ml_dtypes

import concourse.bass as bass
import concourse.bacc as bacc
import concourse.tile as tile
from bass_rust import add_dep_helper
from concourse import mybir
from concourse.bass_utils import run_bass_kernel_spmd

B, T, D = 8, 2048, 512
C = 128
NCH = T // C  # 16
G = 2         # chunks per DMA transfer (in)
NG = NCH // G
SG = 4        # chunks per DMA transfer (out)
FP32 = mybir.dt.float32
BF16 = mybir.dt.bfloat16
BF = ml_dtypes.bfloat16

_prog_cache = {}


def _build_program(n_prev):
    nc = bacc.Bacc(
        "TRN2", target_bir_lowering=False, debug=False, num_devices=B
    )
    LW = (n_prev + 1) * C  # decay lhsT width
    CW = LW + C + 36 + 4 + 49  # + m0 + uta + utc + utb
    h_d = nc.dram_tensor("h", [C, NCH * D], BF16, kind="ExternalInput").ap()
    cst_d = nc.dram_tensor("cst", [C, CW], BF16, kind="ExternalInput").ap()
    k2_d = nc.dram_tensor("k2", [33, C], BF16, kind="ExternalInput").ap()
    bnd_d = nc.dram_tensor(
        "bnd", [1, (NCH - 1) * D], BF16, kind="ExternalInput"
    ).ap()
    out_d = nc.dram_tensor(
        "out", [C, NCH * D], BF16, kind="ExternalOutput"
    ).ap()

    Sqrt = mybir.ActivationFunctionType.Sqrt
    Abs = mybir.ActivationFunctionType.Abs

    with tile.TileContext(nc) as tc:
        with (
            tc.tile_pool(name="const", bufs=1) as const,
            tc.tile_pool(name="hbuf", bufs=1) as hbuf,
            tc.tile_pool(name="rsb", bufs=1) as rsb,
            tc.tile_pool(name="small", bufs=12) as small,
            tc.tile_pool(name="outp", bufs=1) as outp,
            tc.tile_pool(name="psr", bufs=1, space="PSUM") as psr,
            tc.tile_pool(name="psc", bufs=3, space="PSUM") as psc,
            tc.tile_pool(name="psd", bufs=3, space="PSUM") as psd,
        ):
            cst_sb = const.tile([C, CW], BF16, tag="cst")
            nc.sync.dma_start(out=cst_sb[:], in_=cst_d)

            # trigger both ACT table-set loads while the engines are idle
            tiny = const.tile([1, 1], FP32, tag="tiny")
            nc.vector.memset(tiny[:], 1.0)
            nc.scalar.activation(out=tiny[:], in_=tiny[:], func=Abs)
            nc.scalar.activation(out=tiny[:], in_=tiny[:], func=Sqrt)

            dec_sb = cst_sb[:, 0:LW]
            m0_sb = cst_sb[:, LW:LW + C]
            uta_sb = cst_sb[:, LW + C:LW + C + 36]
            utc_sb = cst_sb[:, LW + C + 36:LW + C + 40]
            utb_sb = cst_sb[:, LW + C + 40:LW + C + 89]

            # rc_all holds per-chunk K=2 rhs: partition 0 = R[n] (carry),
            # partition 1 = h[n*C-1] (boundary row), free slot n.
            rc_all = rsb.tile([2, NCH, D], BF16, tag="rc")

            # prefix groups: A1 -> R[1..6], A2 -> R[7..8], B -> R[9..15]
            ra1_t = psr.tile([6, D], FP32, tag="ra1", name="ra1_t")
            ra1_ps = ra1_t[:]
            ra2_t = psr.tile([2, D], FP32, tag="ra2", name="ra2_t")
            ra2_ps = ra2_t[:]
            # B reuses A1's bank (A1 is released by its copy before the
            # first B mask runs)
            rb_t = psr.tile([7, D], FP32, tag="ra1", name="rb_t")
            rb_ps = rb_t[:]

            ht = hbuf.tile([C, NCH, D], BF16, tag="h")
            obuf = outp.tile([C, NCH, D], BF16, tag="o")

            def h_sb(n):
                return ht[:, n, :]

            k2_sb = None
            del_tiles = {}
            rs = rsb.tile([71, D], BF16, tag="rs")

            # first two chunks load individually (smaller bus transfer
            # -> the first prefix matmul starts sooner); rest in pairs
            LOADS = [(c, 2) for c in range(0, NCH, 2)]
            for gi, (c0, cn) in enumerate(LOADS):
                src = h_d[:, c0 * D:(c0 + cn) * D]
                nc.sync.dma_start(
                    out=ht[:, c0:c0 + cn, :],
                    in_=src.rearrange("p (c d) -> p c d", c=cn),
                )
                g = c0 // G
                if gi == 0:
                    # small late-use transfers on the Pool/SWDGE queue:
                    # they bypass the shared HWDGE unit so the bulk h
                    # loads stream back-to-back
                    k2_sb = const.tile([33, C], BF16, tag="k2")
                    nc.scalar.dma_start(out=k2_sb[:], in_=k2_d)
                    nc.scalar.dma_start(
                        out=rc_all[1:2, 1:NCH, :],
                        in_=bnd_d.rearrange("p (c d) -> p c d", c=NCH - 1),
                    )
                if (c0, cn) == (6, 2):
                    # A1 -> SBUF: rs[0]=R[6], rs[1:6]=R[1..5]; strip to rc
                    nc.vector.tensor_copy(rs[0:6, :], ra1_ps)
                    nc.scalar.dma_start(
                        out=rc_all[0:1, 1:6, :], in_=rs[1:6, :]
                    )
                    nc.scalar.dma_start(
                        out=rc_all[0:1, 6:7, :], in_=rs[0:1, :]
                    )
                if (c0, cn) == (8, 2):
                    # seed A2 with R[6] read directly from rs[0] (base 0)
                    nc.tensor.matmul(
                        ra2_ps, k2_sb[0:1, 0:2], rs[0:1, :],
                        start=False, stop=True,
                    )
                    # A2 -> SBUF: rs[32]=R[8], rs[33]=R[7]; strip to rc
                    nc.vector.tensor_copy(rs[32:34, :], ra2_ps)
                    nc.scalar.dma_start(
                        out=rc_all[0:1, 7:8, :], in_=rs[33:34, :]
                    )
                    nc.scalar.dma_start(
                        out=rc_all[0:1, 8:9, :], in_=rs[32:33, :]
                    )
                for c in range(c0, c0 + cn):
                    rhs = h_sb(c)
                    if c <= 5:
                        nc.tensor.matmul(
                            ra1_ps, uta_sb[:, c * 6:(c + 1) * 6], rhs,
                            start=(c == 0), stop=(c == 5),
                        )
                    if 6 <= c <= 7:
                        mm = nc.tensor.matmul(
                            ra2_ps, utc_sb[:, (c - 6) * 2:(c - 5) * 2],
                            rhs, start=(c == 6), stop=False,
                        )
                        last_a_mm = mm
                    if 8 <= c <= 14:
                        nc.tensor.matmul(
                            rb_ps, utb_sb[:, (c - 8) * 7:(c - 7) * 7],
                            rhs, start=(c == 8), stop=False,
                        )
            # M0 matmuls: emitted after the pair loop so the prefix
            # matmuls (which gate the carry chain) always win PE priority;
            # the scheduler backfills these into PE idle slots
            for c in range(NCH):
                del_ps = psd.tile([C, D], FP32, tag="del_ps", name=f"del{c}")
                mm = nc.tensor.matmul(
                    del_ps[:], m0_sb, h_sb(c), start=True, stop=(c == 0)
                )
                # scheduling-only edge: keep the PE clear for the prefix
                # matmuls (they gate the carry chain -> all delta epilogues)
                add_dep_helper(mm.ins, last_a_mm.ins,
                               sync=False, reason="prefix-A first")
                del_tiles[c] = del_ps

            # seed B with R[8] read directly from rs[32] (base 32)
            nc.tensor.matmul(
                rb_ps, k2_sb[32:33, 0:7], rs[32:33, :],
                start=False, stop=True,
            )
            nc.vector.tensor_copy(rs[64:71, :], rb_ps)
            nc.scalar.dma_start(out=rc_all[0:1, 9:16, :], in_=rs[64:71, :])

            # single interleaved loop: the carry-gated delta chain (K2 ->
            # ttr -> sqrt) and the carry-independent context chain
            # (ctx matmuls -> fused evac) are emitted chunk-interleaved so
            # each engine always has lower-priority ready work while the
            # carry is in flight
            STORES = {3: (0, 4), 7: (4, 4), 11: (8, 4), 13: (12, 2),
                      14: (14, 1), 15: (15, 1)}
            for n in range(NCH):
                del_ps = del_tiles[n]
                if n > 0:
                    nc.tensor.matmul(
                        del_ps[:], k2_sb[0:2, :], rc_all[:, n, :],
                        start=False, stop=True,
                    )
                # delta_mag^2 = D * (mean^2 + var) via bn_stats
                stats = small.tile([C, 6], FP32, tag="stats")
                nc.vector.bn_stats(stats[:], del_ps[:])
                mv = small.tile([C, 2], FP32, tag="mv")
                nc.vector.bn_aggr(mv[:], stats[:])
                ex2 = small.tile([C, 1], FP32, tag="ex2")
                nc.vector.tensor_mul(ex2[:], mv[:, 0:1], mv[:, 0:1])
                nc.vector.tensor_add(ex2[:], ex2[:], mv[:, 1:2])
                mag = small.tile([C, 1], FP32, tag="mag")
                nc.scalar.activation(
                    out=mag[:], in_=ex2[:], func=Sqrt, scale=float(D)
                )
                # context: banded decay matmuls + Abs evac + mag multiply
                ctx_ps = psc.tile([C, D], FP32, tag="ctx_ps")
                lags = list(range(0, min(n, n_prev) + 1))
                for l in lags:
                    mm = nc.tensor.matmul(
                        ctx_ps[:], dec_sb[:, l * C:(l + 1) * C], h_sb(n - l),
                        start=(l == lags[0]), stop=(l == lags[-1]),
                    )
                    add_dep_helper(mm.ins, last_a_mm.ins,
                                   sync=False, reason="prefix first")
                o4 = obuf[:, n, :]
                nc.scalar.activation(out=o4, in_=ctx_ps[:], func=Abs)
                eng = nc.gpsimd if n < 8 else nc.vector
                eng.tensor_scalar(
                    o4, o4, mag[:, 0:1], None, mybir.AluOpType.mult
                )
                if n in STORES:
                    s0, sn = STORES[n]
                    dst = out_d[:, s0 * D:(s0 + sn) * D]
                    nc.sync.dma_start(
                        out=dst.rearrange("p (c d) -> p c d", c=sn),
                        in_=obuf[:, s0:s0 + sn, :],
                    )

    nc.compile()
    return nc


def _host_consts(kval):
    # softplus in fp32, matching jax.nn.softplus numerics closely
    kf = np.float32(np.asarray(kval))
    s = np.float32(np.logaddexp(np.float32(0.0), kf))
    # distance beyond which exp(-s*d) is exactly 0 in fp32 (incl. subnormals)
    dmax = 103.98 / float(s)
    n_prev = int(min(NCH - 1, (dmax + C - 1) // C))

    i = np.arange(C)
    j = np.arange(C)
    dec = np.zeros((C, (n_prev + 1) * C), dtype=np.float32)
    for l in range(n_prev + 1):
        dist = (l * C + i[None, :] - j[:, None]).astype(np.float32)
        mask = dist >= 0
        tilew = np.exp((-s) * np.maximum(dist, 0)).astype(np.float32)
        tilew = np.where(mask, tilew, np.float32(0))  # [j, i]
        dec[:, l * C:(l + 1) * C] = tilew

    m0 = ((j[:, None] <= i[None, :]) & (j[:, None] != i[None, :] - 1)).astype(
        np.float32
    )  # [j, i] lhsT layout

    k2 = np.zeros((33, C), dtype=np.float32)
    k2[0, :] = 1.0
    k2[1, 0] = -1.0
    k2[32, :] = 1.0  # ones lhsT at base partition 32 (B seed)

    # prefix groups chained by K=1 seeds:
    # A1 rows R[1..6]; A2 rows R[7..8] = R[6] + partial; B rows R[9..15]
    # = R[8] + partial
    # A1 rows: m=0 -> R[6], m=1..5 -> R[1..5] (R[6] at partition 0 so the
    # A2 seed can read it straight from the SBUF copy)
    uta = np.zeros((C, 6 * 6), dtype=np.float32)
    for c in range(6):
        for m in range(6):
            r = 6 if m == 0 else m  # row m holds R[r] = sum_{c<r} S[c]
            if c < r:
                uta[:, c * 6 + m] = 1.0
    # A2 rows: m=0 -> R[8], m=1 -> R[7] (R[8] at partition 32 of the copy)
    utc = np.zeros((C, 2 * 2), dtype=np.float32)
    for ci, c in enumerate((6, 7)):
        for m in range(2):
            r = 8 - m
            if c < r:  # plus seed R[6]
                utc[:, ci * 2 + m] = 1.0
    utb = np.zeros((C, 7 * 7), dtype=np.float32)
    for ci, c in enumerate(range(8, 15)):
        for m in range(7):
            if c <= 8 + m:  # R[9+m] = R[8] + sum_{8<=c<=8+m} S[c]
                utb[:, ci * 7 + m] = 1.0

    cst = np.concatenate([dec, m0, uta, utc, utb], axis=1)
    return n_prev, {"cst": cst.astype(BF), "k2": k2.astype(BF)}


def _run(h, k, trace=False):
    n_prev, consts = _host_consts(k)
    if n_prev not in _prog_cache:
        _prog_cache[n_prev] = _build_program(n_prev)
    nc = _prog_cache[n_prev]
    h = np.ascontiguousarray(np.asarray(h, dtype=np.float32))
    hb = h.astype(BF)  # [B, T, D] bf16
    # pre-transpose to partition-major [C, NCH*D] per batch
    ht = np.ascontiguousarray(
        hb.reshape(B, NCH, C, D).transpose(0, 2, 1, 3).reshape(B, C, NCH * D)
    )
    # boundary rows h[n*C-1] for n=1..NCH-1, flattened on one partition
    bnd = np.ascontiguousarray(
        hb[:, C - 1:T - 1:C, :].reshape(B, 1, (NCH - 1) * D)
    )
    in_maps = [
        {"h": ht[b], "bnd": bnd[b], **consts} for b in range(B)
    ]
    res = run_bass_kernel_spmd(nc, in_maps, list(range(B)), trace=trace)
    out = np.stack(
        [
            res.results[b]["out"]
            .reshape(C, NCH, D)
            .transpose(1, 0, 2)
            .reshape(T, D)
            .astype(np.float32)
            for b in range(B)
        ],
        axis=0,
    )
    return out, res


def kernel(h, k):
    out, _ = _run(h, k, trace=False)
    return out
